# revision 1
# baseline (speedup 1.0000x reference)
"""AudioFinder Trainium2 kernel.

Data parallel over batch: 16 samples -> 8 cores x 2 samples.

Per-sample pipeline (all on one NeuronCore):
  1. 4-layer gated conv encoder on search (T=8192 -> 2040) and query
     (T=2048 -> 504), conv taps as PSUM-accumulated f32r matmuls over
     Cin=80, gated tanh*sigmoid on ACT engine, 1x1 convs + residuals.
  2. VQ: scores s[t,k] = enc_s[t]@emb[k] - |emb[k]|^2/2 via one f32r
     matmul per 128-t block (enc block stationary).  The nearest-codeword
     classifier contribution is recovered value-space: for j in {0,1}
       u_j[t] = max_k (s[t,k] + ew[k,j]/BIG),  m[t] = max_k s[t,k]
       => (u_j - m)*BIG = ew[argmax_k s, j]   (exact up to fp32 rounding)
     where ew = emb @ w_lin.T.  No argmax/gather instructions needed.
  3. out[s,j] = tanh(max_t((u_j[t]-m[t])*BIG + v_j[t mod 504]) + b_lin[j])
     with v = enc_q @ w_lin.T (tiled x4 + zero pad, -inf beyond T).
"""

import numpy as np

import concourse.bacc as bacc
import concourse.mybir as mybir
import concourse.tile as tile
from concourse.bass_utils import run_bass_kernel_spmd

F32 = mybir.dt.float32
F32R = mybir.dt.float32r
AF = mybir.ActivationFunctionType
OP = mybir.AluOpType
AX = mybir.AxisListType

NCORES = 8
SPC = 2          # samples per core
C = 80
NK = 512         # codebook size
BIG = 1024.0
NEG = -1e30
CH = 512         # chunk (free-dim) size

# layer geometry: (T0h, T1, E1, O1, T2, T3, T4)
GEO_SEARCH = dict(T0h=4096, T1=4095, E1=2048, O1=2047, T2=2046, T3=2043, T4=2040)
GEO_QUERY = dict(T0h=1024, T1=1023, E1=512, O1=511, T2=510, T3=507, T4=504)

# wpack column offsets
def _w_off(kind, i, j=0):
    if kind == "a":
        return (i * 4 + j) * C
    if kind == "g":
        return 1280 + (i * 4 + j) * C
    if kind == "1":
        return 2560 + i * C
    if kind == "f0":
        return 2880
    if kind == "f1":
        return 2960
    raise KeyError(kind)


M_F1 = 82  # f1 conv emits 80 real channels + const-1 channel + zero channel


WPACK_COLS = 3042
# bias pack columns: ba0..3, bg0..3, b10..3, bf0, bf1
def _b_off(kind, i=0):
    return {"a": i, "g": 4 + i, "1": 8 + i, "f0": 12, "f1": 13}[kind]


DEBUG_TAPS = False


def _build():
    nc = bacc.Bacc("TRN2", target_bir_lowering=False, debug=False,
                   num_devices=NCORES)
    d_se = nc.dram_tensor("se", [SPC, C, 4096], F32, kind="ExternalInput")
    d_so = nc.dram_tensor("so", [SPC, C, 4096], F32, kind="ExternalInput")
    d_qe = nc.dram_tensor("qe", [SPC, C, 1024], F32, kind="ExternalInput")
    d_qo = nc.dram_tensor("qo", [SPC, C, 1024], F32, kind="ExternalInput")
    d_wpk = nc.dram_tensor("wpk", [C, WPACK_COLS], F32, kind="ExternalInput")
    d_bpk = nc.dram_tensor("bpk", [M_F1, 14], F32, kind="ExternalInput")
    d_epk3 = nc.dram_tensor("epk3", [M_F1, 3 * NK], F32, kind="ExternalInput")
    d_wlt = nc.dram_tensor("wlt", [C, 2], F32, kind="ExternalInput")
    d_blt = nc.dram_tensor("blt", [4, 1], F32, kind="ExternalInput")
    d_out = nc.dram_tensor("out", [SPC, 2], F32, kind="ExternalOutput")
    d_zbuf = nc.dram_tensor("zbuf", [3 * SPC, 2048], F32)
    dbg = {}
    if DEBUG_TAPS:
        dbg["encs"] = nc.dram_tensor("dbg_encs", [SPC, C, 2040], F32,
                                     kind="ExternalOutput")
        dbg["encq"] = nc.dram_tensor("dbg_encq", [SPC, C, 504], F32,
                                     kind="ExternalOutput")
        dbg["m"] = nc.dram_tensor("dbg_m", [SPC, 128, 16], F32,
                                  kind="ExternalOutput")
        dbg["u0"] = nc.dram_tensor("dbg_u0", [SPC, 128, 16], F32,
                                   kind="ExternalOutput")
        dbg["z"] = nc.dram_tensor("dbg_z", [4, 2048], F32,
                                  kind="ExternalOutput")

    with tile.TileContext(nc) as tc:
        with (
            tc.tile_pool(name="sb", bufs=1) as sb,
            tc.tile_pool(name="ps", bufs=2, space="PSUM") as ps,
        ):
            # ---- static tables ----
            wpks = sb.tile([C, WPACK_COLS], F32, tag="wstage")
            nc.sync.dma_start(wpks[:], d_wpk[:])
            wpk = sb.tile([C, WPACK_COLS], F32R, tag="wpk")
            nc.vector.tensor_copy(wpk[:], wpks[:])  # round to f32r
            bpk = sb.tile([M_F1, 14], F32, tag="bpk")
            nc.sync.dma_start(bpk[:], d_bpk[:])
            epks = sb.tile([M_F1, 3 * NK], F32, tag="estage")
            nc.sync.dma_start(epks[:], d_epk3[:])
            epk3 = sb.tile([M_F1, 3 * NK], F32R, tag="epk3")
            nc.vector.tensor_copy(epk3[:], epks[:])
            wlt = sb.tile([C, 2], F32, tag="wlt")
            nc.sync.dma_start(wlt[:], d_wlt[:])
            blt = sb.tile([4, 1], F32, tag="blt")
            nc.sync.dma_start(blt[:], d_blt[:])

            VT4 = sb.tile([4, 2048], F32, tag="vt4")
            U4 = sb.tile([4, 2048], F32, tag="u4")
            M4 = sb.tile([4, 2048], F32, tag="m4")
            nc.vector.memset(VT4[:], NEG)

            def wsl(kind, i, j=0):
                off = _w_off(kind, i, j)
                return wpk[:, off:off + C]

            def bap(kind, i=0):
                o = _b_off(kind, i)
                n = M_F1 if kind == "f1" else C
                return bpk[:n, o:o + 1]

            def wide_layer(i, taps, T_out, write_out):
                """taps: list of 4 (tile_ap, base_off).

                fp32r matmuls need an even moving free dim: odd tail chunks
                are padded by one column (reads land in the +8 pad columns
                of the input tiles; the extra output column is discarded).
                """
                for c0 in range(0, T_out, CH):
                    N = min(CH, T_out - c0)
                    Nmm = N + (N & 1)
                    aps = ps.tile([C, Nmm], F32, tag="aps")
                    gps = ps.tile([C, Nmm], F32, tag="gps")
                    for half, pt in (("a", aps), ("g", gps)):
                        for j, (src, off) in enumerate(taps):
                            nc.tensor.matmul(
                                pt[:], wsl(half, i, j),
                                src[:, off + c0: off + c0 + Nmm],
                                start=(j == 0), stop=(j == 3))
                    ta = sb.tile([C, Nmm], F32, tag="ta", bufs=2)
                    sg = sb.tile([C, Nmm], F32, tag="sg", bufs=2)
                    nc.scalar.activation(ta[:], aps[:], AF.Tanh, bias=bap("a", i))
                    nc.scalar.activation(sg[:], gps[:], AF.Sigmoid, bias=bap("g", i))
                    x2 = sb.tile([C, Nmm], F32R, tag="x2", bufs=2)
                    nc.gpsimd.tensor_mul(x2[:], ta[:], sg[:])
                    xps = ps.tile([C, Nmm], F32, tag="xps")
                    nc.tensor.matmul(xps[:], wsl("1", i), x2[:],
                                     start=True, stop=True)
                    write_out(c0, N, xps[:, :N])

            def emit_sample(s, g, is_query, vq_cb=None):
                """Returns enc tile ([80, T4], F32R for search / F32 query)."""
                T0h, T1 = g["T0h"], g["T1"]
                E1, O1 = g["E1"], g["O1"]
                T2, T3, T4 = g["T2"], g["T3"], g["T4"]
                d_e, d_o = (d_qe, d_qo) if is_query else (d_se, d_so)

                x0e = sb.tile([C, T0h + 8], F32R, tag="x0e")
                x0o = sb.tile([C, T0h + 8], F32R, tag="x0o")
                # zero the pad columns (f32r memset is not a legal ISA op:
                # write via DVE mult-by-0 of an already-loaded f32 tile)
                nc.vector.tensor_scalar(x0e[:, T0h:], bpk[:C, 0:8], 0.0, None,
                                        op0=OP.mult)
                nc.vector.tensor_scalar(x0o[:, T0h:], bpk[:C, 0:8], 0.0, None,
                                        op0=OP.mult)
                for dst, src in ((x0e, d_e), (x0o, d_o)):
                    for c0 in range(0, T0h, 2048):
                        n = min(2048, T0h - c0)
                        stg = sb.tile([C, n], F32, tag="xstage", bufs=2)
                        nc.sync.dma_start(stg[:], src[s, :, c0:c0 + n])
                        nc.gpsimd.tensor_copy(dst[:, c0:c0 + n], stg[:])
                x0e_r = x0e[:]
                x0o_r = x0o[:]

                x1e = sb.tile([C, E1 + 8], F32R, tag="x1e")
                x1o = sb.tile([C, O1 + 8], F32R, tag="x1o")

                def w0(c0, N, xps):
                    ne, no = (N + 1) // 2, N // 2
                    h = c0 // 2
                    nc.vector.tensor_scalar(
                        x1e[:, h:h + ne], xps[:, 0:N:2], bap("1", 0), None,
                        op0=OP.add)
                    nc.vector.tensor_scalar(
                        x1o[:, h:h + no], xps[:, 1:N:2], bap("1", 0), None,
                        op0=OP.add)

                wide_layer(0, [(x0e_r, 0), (x0o_r, 0), (x0e_r, 1), (x0o_r, 1)],
                           T1, w0)

                x2f = sb.tile([C, T2 + 8], F32R, tag="x2f")
                nc.vector.tensor_scalar(x2f[:, T2:], bpk[:C, 0:8], 0.0, None,
                                        op0=OP.mult)

                def w1(c0, N, xps):
                    nc.vector.scalar_tensor_tensor(
                        out=x2f[:, c0:c0 + N], in0=xps, scalar=bap("1", 1),
                        in1=x1o[:, c0 + 1:c0 + 1 + N].bitcast(F32),
                        op0=OP.add, op1=OP.add)

                wide_layer(1, [(x1e[:], 0), (x1o[:], 0), (x1e[:], 1),
                               (x1o[:], 1)], T2, w1)

                x3f = sb.tile([C, T3 + 8], F32R, tag="x3f")

                def w2(c0, N, xps):
                    nc.vector.scalar_tensor_tensor(
                        out=x3f[:, c0:c0 + N], in0=xps, scalar=bap("1", 2),
                        in1=x2f[:, c0 + 3:c0 + 3 + N].bitcast(F32),
                        op0=OP.add, op1=OP.add)

                wide_layer(2, [(x2f[:], 0), (x2f[:], 1), (x2f[:], 2),
                               (x2f[:], 3)], T3, w2)

                x4f = sb.tile([C, T4 + 8], F32R, tag="x4f")

                def w3(c0, N, xps):
                    nc.vector.scalar_tensor_tensor(
                        out=x4f[:, c0:c0 + N], in0=xps, scalar=bap("1", 3),
                        in1=x3f[:, c0 + 3:c0 + 3 + N].bitcast(F32),
                        op0=OP.add, op1=OP.add)

                wide_layer(3, [(x3f[:], 0), (x3f[:], 1), (x3f[:], 2),
                               (x3f[:], 3)], T4, w3)

                # final head: f1(relu(f0(x))); f1 emits M_F1=82 rows where
                # row 80 = 1.0 (zero weights, bias 1) and row 81 = 0 -- the
                # const rows let VQ matmuls add per-codeword offsets.
                enc = sb.tile([M_F1, T4], F32 if is_query else F32R,
                              tag="encq" if is_query else "encs", bufs=2)
                for c0 in range(0, T4, CH):
                    N = min(CH, T4 - c0)
                    p0 = ps.tile([C, N], F32, tag="xps")
                    nc.tensor.matmul(p0[:], wsl("f0", 0), x4f[:, c0:c0 + N],
                                     start=True, stop=True)
                    xf = sb.tile([C, N], F32R, tag="xf", bufs=2)
                    nc.scalar.activation(xf[:], p0[:], AF.Relu, bias=bap("f0"))
                    p1 = ps.tile([M_F1, N], F32, tag="xps")
                    nc.tensor.matmul(p1[:], wpk[:, 2960:2960 + M_F1], xf[:],
                                     start=True, stop=True)
                    nc.scalar.activation(enc[:, c0:c0 + N], p1[:], AF.Identity,
                                         bias=bap("f1"))
                    if vq_cb is not None:
                        vq_cb(c0, N, enc)
                return enc

            out_dmas = []
            for s in range(SPC):
                # VQ result accumulators
                mt = sb.tile([128, 16], F32, tag=f"mt{s}")
                u0t = sb.tile([128, 16], F32, tag=f"u0t{s}")
                u1t = sb.tile([128, 16], F32, tag=f"u1t{s}")
                nc.vector.memset(mt[:], NEG)
                nc.vector.memset(u0t[:], NEG)
                nc.vector.memset(u1t[:], NEG)

                T4 = GEO_SEARCH["T4"]

                def vq_blocks(c0, N, enc, _s=s, _mt=mt, _u0t=u0t, _u1t=u1t):
                    b0 = (c0 + 127) // 128
                    b1 = (c0 + N) // 128 if c0 + N < T4 else 16
                    for b in range(b0, b1):
                        t0 = 128 * b
                        P = min(128, T4 - t0)
                        for ti, tgt in ((0, _mt), (1, _u0t), (2, _u1t)):
                            sps = ps.tile([P, NK], F32, tag="vq")
                            nc.tensor.matmul(
                                sps[:], enc[:, t0:t0 + P],
                                epk3[:, NK * ti:NK * (ti + 1)],
                                start=True, stop=True)
                            nc.vector.tensor_reduce(
                                tgt[:P, b:b + 1], sps[:], axis=AX.X,
                                op=OP.max)

                enc_s = emit_sample(s, GEO_SEARCH, False, vq_cb=vq_blocks)
                enc_q = emit_sample(s, GEO_QUERY, True)

                # v = w_lin @ enc_q  (fp32)
                vps = ps.tile([2, 504], F32, tag="xps")
                nc.tensor.matmul(vps[:], wlt[:], enc_q[:C, :504],
                                 start=True, stop=True)
                vsb = sb.tile([2, 504], F32, tag="vsb", bufs=2)
                nc.scalar.activation(vsb[:], vps[:], AF.Copy)
                for j in range(2):
                    r = 2 * s + j
                    for k in range(4):
                        nc.sync.dma_start(
                            VT4[r:r + 1, 504 * k:504 * (k + 1)],
                            vsb[j:j + 1, :])

                # u/m tiles -> row layout ([128,16] -> [1, 2048] t-major)
                # bounce u/m through DRAM to transpose [128,16] -> t-major row
                for zrow, src in ((3 * s, u0t), (3 * s + 1, u1t),
                                  (3 * s + 2, mt)):
                    nc.sync.dma_start(
                        d_zbuf[zrow].rearrange("(b p) -> p b", p=128), src[:])
                for row, zrow in ((2 * s, 3 * s), (2 * s + 1, 3 * s + 1)):
                    nc.sync.dma_start(U4[row:row + 1, :], d_zbuf[zrow])
                    nc.sync.dma_start(M4[row:row + 1, :], d_zbuf[3 * s + 2])
                if DEBUG_TAPS:
                    nc.sync.dma_start(dbg["encs"][s], enc_s[:C, :].bitcast(F32))
                    nc.sync.dma_start(dbg["encq"][s], enc_q[:C, :])
                    nc.sync.dma_start(dbg["m"][s], mt[:])
                    nc.sync.dma_start(dbg["u0"][s], u0t[:])

            # z = (u - m)*BIG + vt ; out = tanh(max_t z + b)
            nc.vector.memset(VT4[:, 2016:2040], 0.0)
            nc.vector.tensor_sub(U4[:], U4[:], M4[:])
            nc.vector.scalar_tensor_tensor(
                out=U4[:], in0=U4[:], scalar=BIG, in1=VT4[:],
                op0=OP.mult, op1=OP.add)
            zmax = sb.tile([4, 1], F32, tag="zmax")
            nc.vector.tensor_reduce(zmax[:], U4[:], axis=AX.X, op=OP.max)
            outv = sb.tile([4, 1], F32, tag="outv")
            nc.scalar.activation(outv[:], zmax[:], AF.Tanh, bias=blt[:])
            nc.sync.dma_start(d_out[:], outv[:])
            if DEBUG_TAPS:
                nc.sync.dma_start(dbg["z"][:], U4[:])

    nc.finalize()
    return nc


_NC_CACHE = None


def _get_nc():
    global _NC_CACHE
    if _NC_CACHE is None:
        _NC_CACHE = _build()
    return _NC_CACHE


def prep_inputs(search, query, w_wide, b_wide, w_1x1, b_1x1, w_f0, b_f0,
                w_f1, b_f1, embedding, w_lin, b_lin):
    """Host-side packing -> list of per-core input maps."""
    f = np.float32
    search = np.asarray(search, f)
    query = np.asarray(query, f)
    se = np.ascontiguousarray(search[:, 0::2, :].transpose(0, 2, 1))
    so = np.ascontiguousarray(search[:, 1::2, :].transpose(0, 2, 1))
    qe = np.ascontiguousarray(query[:, 0::2, :].transpose(0, 2, 1))
    qo = np.ascontiguousarray(query[:, 1::2, :].transpose(0, 2, 1))

    w_wide = np.asarray(w_wide, f)
    cols = []
    for i in range(4):
        for j in range(4):
            cols.append(w_wide[i, :C, :, j].T)
    for i in range(4):
        for j in range(4):
            cols.append(w_wide[i, C:, :, j].T)
    for i in range(4):
        cols.append(np.asarray(w_1x1, f)[i, :, :, 0].T)
    cols.append(np.asarray(w_f0, f)[:, :, 0].T)
    wf1 = np.zeros((C, M_F1), f)
    wf1[:, :C] = np.asarray(w_f1, f)[:, :, 0].T   # cols 80/81 stay zero
    cols.append(wf1)
    wpk = np.ascontiguousarray(np.concatenate(cols, axis=1))
    assert wpk.shape == (C, WPACK_COLS)

    b_wide = np.asarray(b_wide, f)
    bcols = [b_wide[i, :C] for i in range(4)]
    bcols += [b_wide[i, C:] for i in range(4)]
    bcols += [np.asarray(b_1x1, f)[i] for i in range(4)]
    bcols += [np.asarray(b_f0, f), np.asarray(b_f1, f)]
    bpk = np.zeros((M_F1, 14), f)
    bpk[:C] = np.stack(bcols, axis=1)
    bpk[C, _b_off("f1")] = 1.0   # f1 row 80 = 0*x + 1.0 -> const-1 channel

    emb = np.asarray(embedding, f)[0]            # (512, 80)
    e2 = (emb.astype(np.float64) ** 2).sum(1)
    ew = (emb.astype(np.float64) @ np.asarray(w_lin, f).T.astype(np.float64))
    epk3 = np.zeros((M_F1, 3 * NK), f)
    for ti in range(3):
        epk3[:C, NK * ti:NK * (ti + 1)] = emb.T
    epk3[C, 0:NK] = -0.5 * e2
    epk3[C, NK:2 * NK] = -0.5 * e2 + ew[:, 0] / BIG
    epk3[C, 2 * NK:3 * NK] = -0.5 * e2 + ew[:, 1] / BIG
    wlt = np.ascontiguousarray(np.asarray(w_lin, f).T)
    b_lin = np.asarray(b_lin, f)
    blt = np.array([b_lin[0], b_lin[1], b_lin[0], b_lin[1]], f).reshape(4, 1)

    maps = []
    for c in range(NCORES):
        sl = slice(SPC * c, SPC * (c + 1))
        maps.append({
            "se": se[sl], "so": so[sl], "qe": qe[sl], "qo": qo[sl],
            "wpk": wpk, "bpk": bpk, "epk3": epk3, "wlt": wlt, "blt": blt,
        })
    return maps


def kernel(**inputs):
    nc = _get_nc()
    maps = prep_inputs(**inputs)
    res = run_bass_kernel_spmd(nc, maps, core_ids=list(range(NCORES)))
    out = np.concatenate([r["out"] for r in res.results], axis=0)
    return out.astype(np.float32)


if __name__ == "__main__":
    import reference
    inputs = {k: np.asarray(v) for k, v in reference.setup_inputs().items()}
    got = kernel(**inputs)
    print(got)



# revision 2
# speedup vs baseline: 1.0000x; 1.0000x over previous
"""AudioFinder Trainium2 kernel.

Data parallel over batch: 16 samples -> 8 cores x 2 samples.

Per-sample pipeline (all on one NeuronCore, bf16 matmuls / f32 psum):
  1. 4-layer gated conv encoder on query (T=2048 -> 504) then search
     (T=8192 -> 2040), conv taps as PSUM-accumulated bf16 matmuls over
     Cin=80, gated tanh*sigmoid on ACT+Pool engines, 1x1 convs +
     residuals on DVE.  All matmul operands are bf16 (packed host-side);
     PSUM accumulation is f32.
  2. v = w_lin @ enc_q computed right after the query encoder; the
     tiled-x4 + pad row [1,2048] is bounced through DRAM into the VQ
     accumulator layout [128,16] (t = p + 128*b) while the search
     encoder runs.
  3. VQ: scores s[t,k] = enc_s[t]@emb[k] - |emb[k]|^2/2 via one bf16
     matmul per 128-t block (enc block stationary).  enc rows 80/81 are
     const 1.0; epk3 row 80 carries -|e|^2/2 (shared), row 81 carries
     0 / ew0/BIG / ew1/BIG where ew = emb @ w_lin.T.  For j in {0,1}:
       u_j[t] = max_k (s[t,k] + ew[k,j]/BIG),  m[t] = max_k s[t,k]
       => (u_j - m)*BIG = ew[argmax_k s, j]   (fp32-psum exact)
  4. z = (u-m)*BIG + vt in [128,16] layout; max over free dim on DVE,
     then across partitions on Pool; out = tanh(max z + b_lin).
"""

import numpy as np
import ml_dtypes

import concourse.bacc as bacc
import concourse.mybir as mybir
import concourse.tile as tile
from concourse.bass_utils import run_bass_kernel_spmd

F32 = mybir.dt.float32
BF16 = mybir.dt.bfloat16
AF = mybir.ActivationFunctionType
OP = mybir.AluOpType
AX = mybir.AxisListType

NCORES = 8
SPC = 2          # samples per core
C = 80
NK = 512         # codebook size
BIG = 1024.0
NEG = -1e30
CH = 512         # chunk (free-dim) size

# layer geometry: (T0h, T1, E1, O1, T2, T3, T4)
GEO_SEARCH = dict(T0h=4096, T1=4095, E1=2048, O1=2047, T2=2046, T3=2043, T4=2040)
GEO_QUERY = dict(T0h=1024, T1=1023, E1=512, O1=511, T2=510, T3=507, T4=504)

# wpack column offsets
def _w_off(kind, i, j=0):
    if kind == "a":
        return (i * 4 + j) * C
    if kind == "g":
        return 1280 + (i * 4 + j) * C
    if kind == "1":
        return 2560 + i * C
    if kind == "f0":
        return 2880
    if kind == "f1":
        return 2960
    raise KeyError(kind)


M_F1 = 82  # f1 conv emits 80 real channels + two const-1 channels


WPACK_COLS = 3042
# bias pack columns: ba0..3, bg0..3, b10..3, bf0, bf1
def _b_off(kind, i=0):
    return {"a": i, "g": 4 + i, "1": 8 + i, "f0": 12, "f1": 13}[kind]


def _build():
    nc = bacc.Bacc("TRN2", target_bir_lowering=False, debug=False,
                   num_devices=NCORES)
    d_se = nc.dram_tensor("se", [SPC, C, 4096], BF16, kind="ExternalInput")
    d_so = nc.dram_tensor("so", [SPC, C, 4096], BF16, kind="ExternalInput")
    d_qe = nc.dram_tensor("qe", [SPC, C, 1024], BF16, kind="ExternalInput")
    d_qo = nc.dram_tensor("qo", [SPC, C, 1024], BF16, kind="ExternalInput")
    d_wpk = nc.dram_tensor("wpk", [C, WPACK_COLS], BF16, kind="ExternalInput")
    d_bpk = nc.dram_tensor("bpk", [M_F1, 14], F32, kind="ExternalInput")
    d_epk3 = nc.dram_tensor("epk3", [M_F1, 3 * NK], BF16, kind="ExternalInput")
    d_wlt = nc.dram_tensor("wlt", [C, 2], BF16, kind="ExternalInput")
    d_blt = nc.dram_tensor("blt", [1, 4], F32, kind="ExternalInput")
    d_out = nc.dram_tensor("out", [1, 4], F32, kind="ExternalOutput")
    d_zbuf = nc.dram_tensor("zbuf", [2 * SPC, 2048], F32)

    with tile.TileContext(nc) as tc:
        with (
            tc.tile_pool(name="sb", bufs=1) as sb,
            tc.tile_pool(name="ps", bufs=2, space="PSUM") as ps,
        ):
            # ---- static tables (bf16, loaded directly) ----
            wpk = sb.tile([C, WPACK_COLS], BF16, tag="wpk")
            nc.sync.dma_start(wpk[:], d_wpk[:])
            bpk = sb.tile([M_F1, 14], F32, tag="bpk")
            nc.sync.dma_start(bpk[:], d_bpk[:])
            epk3 = sb.tile([M_F1, 3 * NK], BF16, tag="epk3")
            nc.sync.dma_start(epk3[:], d_epk3[:])
            wlt = sb.tile([C, 2], BF16, tag="wlt")
            nc.sync.dma_start(wlt[:], d_wlt[:])
            brow = sb.tile([1, 4], F32, tag="brow")
            nc.sync.dma_start(brow[:], d_blt[:])

            def wsl(kind, i, j=0):
                off = _w_off(kind, i, j)
                return wpk[:, off:off + C]

            def bap(kind, i=0):
                o = _b_off(kind, i)
                n = M_F1 if kind == "f1" else C
                return bpk[:n, o:o + 1]

            def wide_layer(i, taps, T_out, write_out):
                """taps: list of 4 (tile_ap, base_off)."""
                for c0 in range(0, T_out, CH):
                    N = min(CH, T_out - c0)
                    Nmm = N + (N & 1)
                    aps = ps.tile([C, Nmm], F32, tag="aps")
                    gps = ps.tile([C, Nmm], F32, tag="gps")
                    for half, pt in (("a", aps), ("g", gps)):
                        for j, (src, off) in enumerate(taps):
                            nc.tensor.matmul(
                                pt[:], wsl(half, i, j),
                                src[:, off + c0: off + c0 + Nmm],
                                start=(j == 0), stop=(j == 3))
                    ta = sb.tile([C, Nmm], BF16, tag="ta", bufs=2)
                    sg = sb.tile([C, Nmm], BF16, tag="sg", bufs=2)
                    nc.scalar.activation(ta[:], aps[:], AF.Tanh, bias=bap("a", i))
                    nc.scalar.activation(sg[:], gps[:], AF.Sigmoid, bias=bap("g", i))
                    x2 = sb.tile([C, Nmm], BF16, tag="x2", bufs=2)
                    nc.gpsimd.tensor_mul(x2[:], ta[:], sg[:])
                    xps = ps.tile([C, Nmm], F32, tag="xps")
                    nc.tensor.matmul(xps[:], wsl("1", i), x2[:],
                                     start=True, stop=True)
                    write_out(c0, N, xps[:, :N])

            def emit_sample(s, g, is_query, vq_cb=None):
                """Returns enc tile ([M_F1, T4] bf16)."""
                T0h, T1 = g["T0h"], g["T1"]
                E1, O1 = g["E1"], g["O1"]
                T2, T3, T4 = g["T2"], g["T3"], g["T4"]
                d_e, d_o = (d_qe, d_qo) if is_query else (d_se, d_so)

                x0e = sb.tile([C, T0h + 8], BF16, tag="x0e")
                x0o = sb.tile([C, T0h + 8], BF16, tag="x0o")
                nc.vector.memset(x0e[:, T0h:], 0.0)
                nc.vector.memset(x0o[:, T0h:], 0.0)
                for dst, src in ((x0e, d_e), (x0o, d_o)):
                    for c0 in range(0, T0h, 2048):
                        n = min(2048, T0h - c0)
                        nc.sync.dma_start(dst[:, c0:c0 + n], src[s, :, c0:c0 + n])

                x1e = sb.tile([C, E1 + 8], BF16, tag="x1e")
                x1o = sb.tile([C, O1 + 8], BF16, tag="x1o")
                nc.vector.memset(x1e[:, E1:], 0.0)
                nc.vector.memset(x1o[:, O1:], 0.0)

                def w0(c0, N, xps):
                    ne, no = (N + 1) // 2, N // 2
                    h = c0 // 2
                    nc.vector.tensor_scalar(
                        x1e[:, h:h + ne], xps[:, 0:N:2], bap("1", 0), None,
                        op0=OP.add)
                    nc.vector.tensor_scalar(
                        x1o[:, h:h + no], xps[:, 1:N:2], bap("1", 0), None,
                        op0=OP.add)

                wide_layer(0, [(x0e[:], 0), (x0o[:], 0), (x0e[:], 1), (x0o[:], 1)],
                           T1, w0)

                x2f = sb.tile([C, T2 + 8], BF16, tag="x2f")
                nc.vector.memset(x2f[:, T2:], 0.0)

                def w1(c0, N, xps):
                    nc.vector.scalar_tensor_tensor(
                        out=x2f[:, c0:c0 + N], in0=xps, scalar=bap("1", 1),
                        in1=x1o[:, c0 + 1:c0 + 1 + N],
                        op0=OP.add, op1=OP.add)

                wide_layer(1, [(x1e[:], 0), (x1o[:], 0), (x1e[:], 1),
                               (x1o[:], 1)], T2, w1)

                x3f = sb.tile([C, T3 + 8], BF16, tag="x3f")
                nc.vector.memset(x3f[:, T3:], 0.0)

                def w2(c0, N, xps):
                    nc.vector.scalar_tensor_tensor(
                        out=x3f[:, c0:c0 + N], in0=xps, scalar=bap("1", 2),
                        in1=x2f[:, c0 + 3:c0 + 3 + N],
                        op0=OP.add, op1=OP.add)

                wide_layer(2, [(x2f[:], 0), (x2f[:], 1), (x2f[:], 2),
                               (x2f[:], 3)], T3, w2)

                x4f = sb.tile([C, T4 + 8], BF16, tag="x4f")
                nc.vector.memset(x4f[:, T4:], 0.0)

                def w3(c0, N, xps):
                    nc.vector.scalar_tensor_tensor(
                        out=x4f[:, c0:c0 + N], in0=xps, scalar=bap("1", 3),
                        in1=x3f[:, c0 + 3:c0 + 3 + N],
                        op0=OP.add, op1=OP.add)

                wide_layer(3, [(x3f[:], 0), (x3f[:], 1), (x3f[:], 2),
                               (x3f[:], 3)], T4, w3)

                # final head: f1(relu(f0(x))); f1 emits M_F1=82 rows where
                # rows 80/81 = 1.0 (zero weights, bias 1) -- the const rows
                # let VQ matmuls add per-codeword offsets.
                enc = sb.tile([M_F1, T4], BF16,
                              tag="encq" if is_query else "encs", bufs=2)
                for c0 in range(0, T4, CH):
                    N = min(CH, T4 - c0)
                    p0 = ps.tile([C, N], F32, tag="xps")
                    nc.tensor.matmul(p0[:], wsl("f0", 0), x4f[:, c0:c0 + N],
                                     start=True, stop=True)
                    xf = sb.tile([C, N], BF16, tag="xf", bufs=2)
                    nc.scalar.activation(xf[:], p0[:], AF.Relu, bias=bap("f0"))
                    p1 = ps.tile([M_F1, N], F32, tag="xps")
                    nc.tensor.matmul(p1[:], wpk[:, 2960:2960 + M_F1], xf[:],
                                     start=True, stop=True)
                    nc.scalar.activation(enc[:, c0:c0 + N], p1[:], AF.Identity,
                                         bias=bap("f1"))
                    if vq_cb is not None:
                        vq_cb(c0, N, enc)
                return enc

            zred = sb.tile([128, 4], F32, tag="zred")
            for s in range(SPC):
                # ---- query first: v row bounce overlaps search encoder ----
                enc_q = emit_sample(s, GEO_QUERY, True)
                vps = ps.tile([2, 504], F32, tag="vq")
                nc.tensor.matmul(vps[:], wlt[:], enc_q[:C, :504],
                                 start=True, stop=True)
                vrow = sb.tile([2, 2048], F32, tag="vrow", bufs=2)
                for k in range(4):
                    nc.vector.tensor_copy(vrow[:, 504 * k:504 * (k + 1)],
                                          vps[:])
                nc.vector.memset(vrow[:, 2016:2040], 0.0)
                nc.vector.memset(vrow[:, 2040:2048], NEG)
                nc.sync.dma_start(d_zbuf[2 * s:2 * s + 2, :], vrow[:])
                vt0 = sb.tile([128, 16], F32, tag=f"vt{2 * s}")
                vt1 = sb.tile([128, 16], F32, tag=f"vt{2 * s + 1}")
                nc.sync.dma_start(
                    vt0[:], d_zbuf[2 * s].rearrange("(b p) -> p b", p=128))
                nc.sync.dma_start(
                    vt1[:], d_zbuf[2 * s + 1].rearrange("(b p) -> p b", p=128))

                # ---- search encoder with VQ accumulation ----
                mt = sb.tile([128, 16], F32, tag=f"mt{s}")
                u0t = sb.tile([128, 16], F32, tag=f"u0t{s}")
                u1t = sb.tile([128, 16], F32, tag=f"u1t{s}")
                nc.vector.memset(mt[:], NEG)
                nc.vector.memset(u0t[:], NEG)
                nc.vector.memset(u1t[:], NEG)

                T4 = GEO_SEARCH["T4"]

                def vq_blocks(c0, N, enc, _mt=mt, _u0t=u0t, _u1t=u1t):
                    b0 = (c0 + 127) // 128
                    b1 = (c0 + N) // 128 if c0 + N < T4 else 16
                    for b in range(b0, b1):
                        t0 = 128 * b
                        P = min(128, T4 - t0)
                        for ti, tgt in ((0, _mt), (1, _u0t), (2, _u1t)):
                            sps = ps.tile([P, NK], F32, tag="vq")
                            nc.tensor.matmul(
                                sps[:], enc[:, t0:t0 + P],
                                epk3[:, NK * ti:NK * (ti + 1)],
                                start=True, stop=True)
                            nc.vector.tensor_reduce(
                                tgt[:P, b:b + 1], sps[:], axis=AX.X,
                                op=OP.max)

                emit_sample(s, GEO_SEARCH, False, vq_cb=vq_blocks)

                # ---- z = (u - m)*BIG + vt, reduce over free dim ----
                for j, (ut, vt) in enumerate(((u0t, vt0), (u1t, vt1))):
                    zt = sb.tile([128, 16], F32, tag="zt", bufs=2)
                    nc.vector.tensor_sub(zt[:], ut[:], mt[:])
                    nc.vector.scalar_tensor_tensor(
                        out=zt[:], in0=zt[:], scalar=BIG, in1=vt[:],
                        op0=OP.mult, op1=OP.add)
                    nc.vector.tensor_reduce(
                        zred[:, 2 * s + j:2 * s + j + 1], zt[:], axis=AX.X,
                        op=OP.max)

            # cross-partition max on Pool, + bias, tanh, out
            zrow = sb.tile([1, 4], F32, tag="zrow")
            nc.gpsimd.tensor_reduce(zrow[:], zred[:], axis=AX.C, op=OP.max)
            nc.vector.tensor_add(zrow[:], zrow[:], brow[:])
            outv = sb.tile([1, 4], F32, tag="outv")
            nc.scalar.activation(outv[:], zrow[:], AF.Tanh)
            nc.sync.dma_start(d_out[:], outv[:])

    nc.finalize()
    return nc


_NC_CACHE = None


def _get_nc():
    global _NC_CACHE
    if _NC_CACHE is None:
        _NC_CACHE = _build()
    return _NC_CACHE


def prep_inputs(search, query, w_wide, b_wide, w_1x1, b_1x1, w_f0, b_f0,
                w_f1, b_f1, embedding, w_lin, b_lin):
    """Host-side packing -> list of per-core input maps (bf16 operands)."""
    f = np.float32
    bf = ml_dtypes.bfloat16
    search = np.asarray(search, f)
    query = np.asarray(query, f)
    se = np.ascontiguousarray(search[:, 0::2, :].transpose(0, 2, 1)).astype(bf)
    so = np.ascontiguousarray(search[:, 1::2, :].transpose(0, 2, 1)).astype(bf)
    qe = np.ascontiguousarray(query[:, 0::2, :].transpose(0, 2, 1)).astype(bf)
    qo = np.ascontiguousarray(query[:, 1::2, :].transpose(0, 2, 1)).astype(bf)

    w_wide = np.asarray(w_wide, f)
    cols = []
    for i in range(4):
        for j in range(4):
            cols.append(w_wide[i, :C, :, j].T)
    for i in range(4):
        for j in range(4):
            cols.append(w_wide[i, C:, :, j].T)
    for i in range(4):
        cols.append(np.asarray(w_1x1, f)[i, :, :, 0].T)
    cols.append(np.asarray(w_f0, f)[:, :, 0].T)
    wf1 = np.zeros((C, M_F1), f)
    wf1[:, :C] = np.asarray(w_f1, f)[:, :, 0].T   # cols 80/81 stay zero
    cols.append(wf1)
    wpk = np.ascontiguousarray(np.concatenate(cols, axis=1)).astype(bf)
    assert wpk.shape == (C, WPACK_COLS)

    b_wide = np.asarray(b_wide, f)
    bcols = [b_wide[i, :C] for i in range(4)]
    bcols += [b_wide[i, C:] for i in range(4)]
    bcols += [np.asarray(b_1x1, f)[i] for i in range(4)]
    bcols += [np.asarray(b_f0, f), np.asarray(b_f1, f)]
    bpk = np.zeros((M_F1, 14), f)
    bpk[:C] = np.stack(bcols, axis=1)
    bpk[C, _b_off("f1")] = 1.0     # f1 rows 80/81 = 0*x + 1.0 -> const-1
    bpk[C + 1, _b_off("f1")] = 1.0

    emb = np.asarray(embedding, f)[0]            # (512, 80)
    e2 = (emb.astype(np.float64) ** 2).sum(1)
    ew = (emb.astype(np.float64) @ np.asarray(w_lin, f).T.astype(np.float64))
    epk3 = np.zeros((M_F1, 3 * NK), f)
    for ti in range(3):
        epk3[:C, NK * ti:NK * (ti + 1)] = emb.T
        epk3[C, NK * ti:NK * (ti + 1)] = -0.5 * e2
    epk3[C + 1, NK:2 * NK] = ew[:, 0] / BIG
    epk3[C + 1, 2 * NK:3 * NK] = ew[:, 1] / BIG
    epk3 = epk3.astype(bf)
    wlt = np.ascontiguousarray(np.asarray(w_lin, f).T).astype(bf)
    b_lin = np.asarray(b_lin, f)
    blt = np.array([[b_lin[0], b_lin[1], b_lin[0], b_lin[1]]], f)

    maps = []
    for c in range(NCORES):
        sl = slice(SPC * c, SPC * (c + 1))
        maps.append({
            "se": se[sl], "so": so[sl], "qe": qe[sl], "qo": qo[sl],
            "wpk": wpk, "bpk": bpk, "epk3": epk3, "wlt": wlt, "blt": blt,
        })
    return maps


def kernel(**inputs):
    nc = _get_nc()
    maps = prep_inputs(**inputs)
    res = run_bass_kernel_spmd(nc, maps, core_ids=list(range(NCORES)))
    out = np.concatenate([r["out"].reshape(SPC, 2) for r in res.results],
                         axis=0)
    return out.astype(np.float32)


if __name__ == "__main__":
    import reference
    inputs = {k: np.asarray(v) for k, v in reference.setup_inputs().items()}
    got = kernel(**inputs)
    print(got)
